# revision 24
# baseline (speedup 1.0000x reference)
"""Single-head attention on 8 Trainium2 NeuronCores, batch-sharded.

Per core (one batch element b). Host-side layouts make every DMA a large
contiguous read (6KB/partition for x).

Projections (bf16, chunk order 0..3, interleaved with everything else):
  A [Wv|Wq] -> vq tile:  rows 0-63 v^T, rows 64-127 q^T (hi copy)
  B [Wq|Wk] -> qk tile:  rows 0-63 q^T (lo copy), rows 64-127 k^T (hi tiles)
  C [Wk|0]  -> klo tile: rows 0-63 k^T (lo tiles), N=256 per chunk
Within each 512-col chunk c, k-tiles 4c,4c+1 are assigned to the LOW
partition half and 4c+2,4c+3 to the HIGH half, so paired scores can start
right after chunk 0 arrives. Projection psums alternate between the "pj"
and "o" PSUM banks so consecutive groups pipeline (the WAR wait on the
bias-add is covered by the other group's matmuls).

Scores (bf16, ~2x via PE row tiling): each pair (lo k-tile | hi k-tile)
runs as two CONCURRENT K=64 matmuls in array row groups 0-63 / 64-127
(tile_position auto-derived from base partitions). 3 matmuls fill a
[128,1536] psum tile (2 such tiles rotate = 6 banks).

exp: split across TWO engines. ACT handles q-chunk 0 and groups 1,3,5 of
q-chunks 1-3 (exact exp, scale=1/8 folded in, bf16 out). DVE handles
groups 0,2,4 of q-chunks 1-3 with a Schraudolph bit-trick: bf16 bits of
2^y are linear in y, so   bits = round(s_raw * (log2e/8 * 128) + 16248.5)
computed by ONE tensor_scalar (mult+add, f32 psum in, int16 out) IS
exp(s/8) to within ~2% — the int16 tile is bitcast to bf16 for the PV
matmul. Splitting exp removes it as the serial bottleneck (sim rel err
with this mix: 0.84e-2 < 2e-2).

PV (bf16): per k-tile matmul, M=65 (V plus a ones row -> softmax
denominator), accumulated over the 16 k-tiles into a 1-bank psum, PV of
group g trailing exp of group g+1 within the same q-chunk.

V layout: V^T rows of vq are DMA-TRANSPOSED (SBUF->SBUF XBAR) straight
into v65 [128, kt, 68] — no PE transposes anywhere in the kernel.

Epilogue per q-chunk: DVE copy psum->SBUF, DMA out^T [65,512] f32; the
host does the divide-by-denominator and the final transpose (cheap).
"""

import numpy as np

USE_DMA_TRANSPOSE = False

B, S, D, H = 8, 2048, 768, 64
DT = D // 128          # 6 d-tiles
NQ = S // 512          # 4 q-chunks of 512
NK = S // 128          # 16 k-tiles of 128
SCALE = 1.0 / np.sqrt(H).item()
SCH_A = SCALE * np.log2(np.e).item() * 128.0   # Schraudolph slope
SCH_C = 16248.5                                 # Schraudolph offset (tuned)
GROUPS = ((0, 3), (3, 6), (6, 9), (9, 12), (12, 15), (15, 16))
DVE_GROUPS = {(qc, g) for qc in (1, 2, 3) for g in (0, 2, 4)}

_cache = {}


def _kt_of(n):
    """Score-matmul n (0..15) within a q-chunk -> k-tile index."""
    p, e = divmod(n, 2)
    c, i = divmod(p, 2)
    return 4 * c + i + 2 * e


def _build():
    import concourse.mybir as mybir
    import concourse.tile as tile
    from concourse import bacc

    f32 = mybir.dt.float32
    bf16 = mybir.dt.bfloat16
    i16 = mybir.dt.int16
    Exp = mybir.ActivationFunctionType.Exp
    Mult = mybir.AluOpType.mult
    Add = mybir.AluOpType.add

    from concourse.masks import make_identity

    nc = bacc.Bacc(None)
    xp_d = nc.dram_tensor("xp", [128, NQ, 2, DT * 256], bf16, kind="ExternalInput")
    wA_d = nc.dram_tensor("wA", [128, DT * 128], bf16, kind="ExternalInput")
    wB_d = nc.dram_tensor("wB", [128, DT * 128], bf16, kind="ExternalInput")
    wC_d = nc.dram_tensor("wC", [128, DT * 128], bf16, kind="ExternalInput")
    bA_d = nc.dram_tensor("bA", [128, 1], f32, kind="ExternalInput")
    bB_d = nc.dram_tensor("bB", [128, 1], f32, kind="ExternalInput")
    bC_d = nc.dram_tensor("bC", [128, 1], f32, kind="ExternalInput")
    out_d = nc.dram_tensor("out", [H + 1, NQ * 512], f32, kind="ExternalOutput")

    with tile.TileContext(nc) as tc:
        with (
            tc.tile_pool(name="big", bufs=1) as big,
            tc.tile_pool(name="small", bufs=1) as small,
            tc.tile_pool(name="pt", bufs=10) as ptp,
            tc.tile_pool(name="res", bufs=2) as resp,
            tc.tile_pool(name="ps", bufs=2, space="PSUM") as ps,
        ):
            if not USE_DMA_TRANSPOSE:
                ident = small.tile([128, 128], f32)
                make_identity(nc, ident)
                identb = small.tile([128, 128], bf16)
                nc.gpsimd.tensor_copy(out=identb, in_=ident)

            # warm the ACT exp table during DMA fill
            zwarm = small.tile([128, 8], f32)
            nc.gpsimd.memset(zwarm, 0.0)
            wwarm = small.tile([128, 8], bf16)
            nc.scalar.activation(out=wwarm, in_=zwarm, func=Exp)

            # ---- DMAs, ordered so chunk-0 compute starts earliest ----
            wA = small.tile([128, DT, 128], bf16)
            wB = small.tile([128, DT, 128], bf16)
            wC = small.tile([128, DT, 128], bf16)
            bA = small.tile([128, 1], f32)
            bB = small.tile([128, 1], f32)
            bC = small.tile([128, 1], f32)
            xT = big.tile([128, NQ, 2, DT * 256], bf16)

            nc.sync.dma_start(out=xT[:, 0, 0, :], in_=xp_d[:, 0, 0, :])
            nc.sync.dma_start(out=wA, in_=wA_d[:, :].rearrange("p (t h) -> p t h", t=DT))
            nc.sync.dma_start(out=bA, in_=bA_d[:, :])
            nc.sync.dma_start(out=xT[:, 0, 1, :], in_=xp_d[:, 0, 1, :])
            nc.sync.dma_start(out=wB, in_=wB_d[:, :].rearrange("p (t h) -> p t h", t=DT))
            nc.sync.dma_start(out=bB, in_=bB_d[:, :])
            nc.sync.dma_start(out=wC, in_=wC_d[:, :].rearrange("p (t h) -> p t h", t=DT))
            nc.sync.dma_start(out=bC, in_=bC_d[:, :])
            for c in (1, 2, 3):
                for hh in (0, 1):
                    nc.sync.dma_start(out=xT[:, c, hh, :], in_=xp_d[:, c, hh, :])

            # ---- persistent tensors ----
            vq = big.tile([128, S], bf16, tag="vq")    # v^T lo / q^T hi
            qk = big.tile([128, S], bf16, tag="qk")    # q^T lo / k^T hi
            klo = big.tile([H, S], bf16, tag="klo")    # k^T lo tiles
            v65 = big.tile([128, NK, H + 4], bf16, tag="v65")
            nc.gpsimd.memset(v65[:, :, H : H + 1], 1.0)

            # proj psums alternate between two 1-bank tags
            alt = {"n": 0}

            def pj_tile(cols, nm, dtype=f32):
                tag = ("pj", "o")[alt["n"] % 2]
                alt["n"] += 1
                return ps.tile([128, cols], dtype, tag=tag, name=nm, bufs=1)

            pt_tiles = {}
            sc_state = {}
            outqs = [None] * NQ

            def emit_pv(qc, n):
                g = n // 3
                lo, _hi = GROUPS[g]
                pt = pt_tiles[(qc, g)]
                rhs_t = pt if pt.dtype == bf16 else pt.bitcast(bf16)
                slot = n - lo
                nc.tensor.matmul(
                    outqs[qc],
                    lhsT=v65[:, _kt_of(n), : H + 1],
                    rhs=rhs_t[:, slot * 512 : (slot + 1) * 512],
                    start=(n == 0),
                    stop=(n == NK - 1),
                )

            def emit_epilogue(qc):
                oT = resp.tile([H + 1, 512], f32, tag="oT", name=f"oT{qc}")
                nc.vector.tensor_copy(out=oT, in_=outqs[qc])
                nc.sync.dma_start(
                    out=out_d[:, qc * 512 : (qc + 1) * 512], in_=oT
                )

            def emit_score_pair(qc, p):
                """Two concurrent row-tiled matmuls for pair p (lo kt | hi kt)."""
                st = sc_state.setdefault(qc, [None] * len(GROUPS))
                for n in (2 * p, 2 * p + 1):
                    g = n // 3
                    lo, hi = GROUPS[g]
                    if st[g] is None:
                        st[g] = ps.tile(
                            [128, (hi - lo) * 512], f32, tag="sc", name=f"sc{qc}_{g}"
                        )
                    slot = n - lo
                    kt = _kt_of(n)
                    if n % 2 == 0:
                        lhsT = klo[:, kt * 128 : (kt + 1) * 128]
                        rhs = qk[:H, qc * 512 : (qc + 1) * 512]
                    else:
                        lhsT = qk[H:, kt * 128 : (kt + 1) * 128]
                        rhs = vq[H:, qc * 512 : (qc + 1) * 512]
                    nc.tensor.matmul(
                        st[g][:, slot * 512 : (slot + 1) * 512],
                        lhsT=lhsT,
                        rhs=rhs,
                        start=True,
                        stop=True,
                    )
                    if n == hi - 1:  # group full -> exp
                        cols = (hi - lo) * 512
                        if (qc, g) in DVE_GROUPS:
                            pt = ptp.tile(
                                [128, cols], i16, tag="pT", name=f"pt{qc}_{g}"
                            )
                            nc.vector.tensor_scalar(
                                out=pt,
                                in0=st[g],
                                scalar1=SCH_A,
                                scalar2=SCH_C,
                                op0=Mult,
                                op1=Add,
                            )
                        else:
                            pt = ptp.tile(
                                [128, cols], bf16, tag="pT", name=f"pt{qc}_{g}"
                            )
                            nc.scalar.activation(
                                out=pt, in_=st[g], func=Exp, scale=SCALE
                            )
                        pt_tiles[(qc, g)] = pt

            # ---- projection phase (chunk c), with q-chunk-0 scores inline ----
            for c in range(4):
                cc = slice(c * 512, (c + 1) * 512)

                def proj(w, nm, c0=c, halves=(0, 1)):
                    p = pj_tile(256 * len(halves), f"ps{nm}{c0}")
                    for hh in halves:
                        for dt in range(DT):
                            nc.tensor.matmul(
                                p[:, hh * 256 : (hh + 1) * 256],
                                lhsT=w[:, dt, :],
                                rhs=xT[:, c0, hh, dt * 256 : (dt + 1) * 256],
                                start=(dt == 0),
                                stop=(dt == DT - 1),
                            )
                    return p

                psA = proj(wA, "A")
                nc.vector.tensor_scalar_add(out=vq[:, cc], in0=psA, scalar1=bA)
                psB = proj(wB, "B")
                nc.vector.tensor_scalar_add(out=qk[:, cc], in0=psB, scalar1=bB)
                psC = proj(wC, "C", halves=(0,))
                nc.vector.tensor_scalar_add(
                    out=klo[:, c * 512 : c * 512 + 256],
                    in0=psC[:H, :],
                    scalar1=bC[:H, :],
                )
                # V rows -> v65 [k, h] layout
                for j in range(4):
                    kt = 4 * c + j
                    if USE_DMA_TRANSPOSE:
                        nc.sync.dma_start(
                            out=v65[:, kt, :H],
                            in_=vq[:H, kt * 128 : (kt + 1) * 128],
                            transpose=True,
                        )
                    else:
                        tp = pj_tile(128, f"vtr{kt}", dtype=bf16)
                        nc.tensor.transpose(
                            tp[:, :H],
                            vq[:H, kt * 128 : (kt + 1) * 128],
                            identb[:H, :H],
                        )
                        nc.vector.tensor_copy(out=v65[:, kt, :H], in_=tp[:, :H])
                # q-chunk 0 scores for this chunk's two pairs
                emit_score_pair(0, 2 * c)
                emit_score_pair(0, 2 * c + 1)

            # ---- steady phase ----
            # PV(qc) trails exp(qc) by one group within the same q-chunk;
            # PV(0) (whose outq can't be allocated mid-projection) is pumped
            # during qc=1. outq banks alternate between tags "o" and "pj".
            pv_cursor = [0] * NQ

            def pump_pv(qc, limit_n):
                while pv_cursor[qc] < min(limit_n, NK):
                    emit_pv(qc, pv_cursor[qc])
                    pv_cursor[qc] += 1

            outqs[0] = ps.tile([H + 1, 512], f32, tag="o", name="outq0", bufs=1)
            for qc in range(1, NQ):
                for p in range(8):
                    emit_score_pair(qc, p)
                    if qc == 1:
                        pump_pv(0, 2 * p + 2)
                    ge = sum(1 for g in range(6) if (qc, g) in pt_tiles)
                    if ge >= 2:
                        if outqs[qc] is None:
                            outqs[qc] = ps.tile(
                                [H + 1, 512],
                                f32,
                                tag=("o", "pj")[qc % 2],
                                name=f"outq{qc}",
                                bufs=1,
                            )
                        pump_pv(qc, 3 * (ge - 1))
                if qc == 1:
                    pump_pv(0, NK)
                    emit_epilogue(0)
                pump_pv(qc, NK)
                emit_epilogue(qc)

    nc.compile()
    return nc


def _get_nc():
    if "nc" not in _cache:
        _cache["nc"] = _build()
    return _cache["nc"]


def _prep_inputs(x, Wq, bq, Wk, bk, Wv, bv):
    import ml_dtypes

    x = np.asarray(x, np.float32)
    Wq = np.asarray(Wq, np.float32)
    Wk = np.asarray(Wk, np.float32)
    Wv = np.asarray(Wv, np.float32)
    bq = np.asarray(bq, np.float32).ravel()
    bk = np.asarray(bk, np.float32).ravel()
    bv = np.asarray(bv, np.float32).ravel()

    def wprep(w):  # [768,128] -> [128, DT*128]: (p, dt*128+h) = w[dt*128+p, h]
        return np.ascontiguousarray(
            w.reshape(DT, 128, 128).transpose(1, 0, 2).reshape(128, DT * 128)
        ).astype(ml_dtypes.bfloat16)

    z = np.zeros((D, H), np.float32)
    common = {
        "wA": wprep(np.concatenate([Wv, Wq], axis=1)),
        "wB": wprep(np.concatenate([Wq, Wk], axis=1)),
        "wC": wprep(np.concatenate([Wk, z], axis=1)),
        "bA": np.ascontiguousarray(np.concatenate([bv, bq]).reshape(128, 1)),
        "bB": np.ascontiguousarray(np.concatenate([bq, bk]).reshape(128, 1)),
        "bC": np.ascontiguousarray(
            np.concatenate([bk, np.zeros(H, np.float32)]).reshape(128, 1)
        ),
    }
    return x, common


def _xprep(xb):
    """[S, D] f32 -> [128, NQ, 2, DT*256] bf16:
    (p, c, h, dt*256+j) = x[c*512 + h*256 + j, dt*128 + p]"""
    import ml_dtypes

    t = xb.reshape(NQ, 2, 256, DT, 128).transpose(4, 0, 1, 3, 2)
    return np.ascontiguousarray(t.reshape(128, NQ, 2, DT * 256)).astype(
        ml_dtypes.bfloat16
    )


def _unshard_out(o):
    """[65, NQ*512] out^T with denominator row -> [S, H]"""
    o = np.asarray(o, np.float32)
    return (o[:H, :] / o[H : H + 1, :]).T


def _in_maps(x, common):
    return [{"xp": _xprep(x[b]), **common} for b in range(B)]


def kernel(x, Wq, bq, Wk, bk, Wv, bv, **_):
    from concourse.bass_utils import run_bass_kernel_spmd

    nc = _get_nc()
    x, common = _prep_inputs(x, Wq, bq, Wk, bk, Wv, bv)
    res = run_bass_kernel_spmd(nc, _in_maps(x, common), core_ids=list(range(B)))
    return np.stack([_unshard_out(res.results[b]["out"]) for b in range(B)])


# revision 25
# speedup vs baseline: 1.0401x; 1.0401x over previous
"""Single-head attention on 8 Trainium2 NeuronCores, batch-sharded.

Per core (one batch element b). Host-side layouts make every DMA a large
contiguous read (6KB/partition for x).

Projections (bf16, chunk order 0..3, interleaved with everything else):
  A [Wv|Wq] -> vq tile:  rows 0-63 v^T, rows 64-127 q^T (hi copy)
  B [Wq|Wk] -> qk tile:  rows 0-63 q^T (lo copy), rows 64-127 k^T (hi tiles)
  C [Wk|0]  -> klo tile: rows 0-63 k^T (lo tiles), N=256 per chunk
Within each 512-col chunk c, k-tiles 4c,4c+1 are assigned to the LOW
partition half and 4c+2,4c+3 to the HIGH half, so paired scores can start
right after chunk 0 arrives. Projection psums alternate between the "pj"
and "o" PSUM banks so consecutive groups pipeline (the WAR wait on the
bias-add is covered by the other group's matmuls).

Scores (bf16, ~2x via PE row tiling): each pair (lo k-tile | hi k-tile)
runs as two CONCURRENT K=64 matmuls in array row groups 0-63 / 64-127
(tile_position auto-derived from base partitions). 3 matmuls fill a
[128,1536] psum tile (2 such tiles rotate = 6 banks).

exp: split across TWO engines. ACT handles q-chunk 0 and groups 1,3,5 of
q-chunks 1-3 (exact exp, scale=1/8 folded in, bf16 out). DVE handles
groups 0,2,4 of q-chunks 1-3 with a Schraudolph bit-trick: bf16 bits of
2^y are linear in y, so   bits = round(s_raw * (log2e/8 * 128) + 16248.5)
computed by ONE tensor_scalar (mult+add, f32 psum in, int16 out) IS
exp(s/8) to within ~2% — the int16 tile is bitcast to bf16 for the PV
matmul. Splitting exp removes it as the serial bottleneck (sim rel err
with this mix: 0.84e-2 < 2e-2).

PV (bf16): per k-tile matmul, M=65 (V plus a ones row -> softmax
denominator), accumulated over the 16 k-tiles into a 1-bank psum, PV of
group g trailing exp of group g+1 within the same q-chunk.

V layout: V^T rows of vq are DMA-TRANSPOSED (SBUF->SBUF XBAR) straight
into v65 [128, kt, 68] — no PE transposes anywhere in the kernel.

Epilogue per q-chunk: DVE copy psum->SBUF, DMA out^T [65,512] f32; the
host does the divide-by-denominator and the final transpose (cheap).
"""

import numpy as np

USE_DMA_TRANSPOSE = False

B, S, D, H = 8, 2048, 768, 64
DT = D // 128          # 6 d-tiles
NQ = S // 512          # 4 q-chunks of 512
NK = S // 128          # 16 k-tiles of 128
SCALE = 1.0 / np.sqrt(H).item()
SCH_A = SCALE * np.log2(np.e).item() * 128.0   # Schraudolph slope
SCH_C = 16248.5                                 # Schraudolph offset (tuned)
GROUPS = ((0, 3), (3, 6), (6, 9), (9, 12), (12, 15), (15, 16))
DVE_GROUPS = {(qc, g) for qc in (1, 2, 3) for g in (0, 2, 4)}

_cache = {}


def _kt_of(n):
    """Score-matmul n (0..15) within a q-chunk -> k-tile index."""
    p, e = divmod(n, 2)
    c, i = divmod(p, 2)
    return 4 * c + i + 2 * e


def _build():
    import concourse.mybir as mybir
    import concourse.tile as tile
    from concourse import bacc

    f32 = mybir.dt.float32
    bf16 = mybir.dt.bfloat16
    i16 = mybir.dt.int16
    Exp = mybir.ActivationFunctionType.Exp
    Mult = mybir.AluOpType.mult
    Add = mybir.AluOpType.add

    from concourse.masks import make_identity

    nc = bacc.Bacc(None)
    xp_d = nc.dram_tensor("xp", [128, NQ, 2, DT * 256], bf16, kind="ExternalInput")
    wA_d = nc.dram_tensor("wA", [128, DT * 128], bf16, kind="ExternalInput")
    wB_d = nc.dram_tensor("wB", [128, DT * 128], bf16, kind="ExternalInput")
    wC_d = nc.dram_tensor("wC", [128, DT * 128], bf16, kind="ExternalInput")
    bA_d = nc.dram_tensor("bA", [128, 1], f32, kind="ExternalInput")
    bB_d = nc.dram_tensor("bB", [128, 1], f32, kind="ExternalInput")
    bC_d = nc.dram_tensor("bC", [128, 1], f32, kind="ExternalInput")
    out_d = nc.dram_tensor("out", [H + 1, NQ * 512], f32, kind="ExternalOutput")

    with tile.TileContext(nc) as tc:
        with (
            tc.tile_pool(name="big", bufs=1) as big,
            tc.tile_pool(name="small", bufs=1) as small,
            tc.tile_pool(name="pt", bufs=10) as ptp,
            tc.tile_pool(name="res", bufs=2) as resp,
            tc.tile_pool(name="ps", bufs=2, space="PSUM") as ps,
        ):
            if not USE_DMA_TRANSPOSE:
                ident = small.tile([128, 128], f32)
                make_identity(nc, ident)
                identb = small.tile([128, 128], bf16)
                nc.gpsimd.tensor_copy(out=identb, in_=ident)

            # warm the ACT exp table during DMA fill
            zwarm = small.tile([128, 8], f32)
            nc.gpsimd.memset(zwarm, 0.0)
            wwarm = small.tile([128, 8], bf16)
            nc.scalar.activation(out=wwarm, in_=zwarm, func=Exp)

            # ---- DMAs, ordered so chunk-0 compute starts earliest ----
            wA = small.tile([128, DT, 128], bf16)
            wB = small.tile([128, DT, 128], bf16)
            wC = small.tile([128, DT, 128], bf16)
            bA = small.tile([128, 1], f32)
            bB = small.tile([128, 1], f32)
            bC = small.tile([128, 1], f32)
            xT = big.tile([128, NQ, 2, DT * 256], bf16)

            nc.sync.dma_start(out=wA, in_=wA_d[:, :].rearrange("p (t h) -> p t h", t=DT))
            nc.sync.dma_start(out=bA, in_=bA_d[:, :])
            nc.sync.dma_start(out=xT[:, 0, 0, :], in_=xp_d[:, 0, 0, :])
            nc.sync.dma_start(out=xT[:, 0, 1, :], in_=xp_d[:, 0, 1, :])
            nc.sync.dma_start(out=wB, in_=wB_d[:, :].rearrange("p (t h) -> p t h", t=DT))
            nc.sync.dma_start(out=bB, in_=bB_d[:, :])
            nc.sync.dma_start(out=wC, in_=wC_d[:, :].rearrange("p (t h) -> p t h", t=DT))
            nc.sync.dma_start(out=bC, in_=bC_d[:, :])
            for c in (1, 2, 3):
                for hh in (0, 1):
                    nc.sync.dma_start(out=xT[:, c, hh, :], in_=xp_d[:, c, hh, :])

            # ---- persistent tensors ----
            vq = big.tile([128, S], bf16, tag="vq")    # v^T lo / q^T hi
            qk = big.tile([128, S], bf16, tag="qk")    # q^T lo / k^T hi
            klo = big.tile([H, S], bf16, tag="klo")    # k^T lo tiles
            v65 = big.tile([128, NK, H + 4], bf16, tag="v65")
            nc.gpsimd.memset(v65[:, :, H : H + 1], 1.0)

            # proj psums alternate between two 1-bank tags
            alt = {"n": 0}

            def pj_tile(cols, nm, dtype=f32):
                tag = ("pj", "o")[alt["n"] % 2]
                alt["n"] += 1
                return ps.tile([128, cols], dtype, tag=tag, name=nm, bufs=1)

            pt_tiles = {}
            sc_state = {}
            outqs = [None] * NQ

            def emit_pv(qc, n):
                g = n // 3
                lo, _hi = GROUPS[g]
                pt = pt_tiles[(qc, g)]
                rhs_t = pt if pt.dtype == bf16 else pt.bitcast(bf16)
                slot = n - lo
                nc.tensor.matmul(
                    outqs[qc],
                    lhsT=v65[:, _kt_of(n), : H + 1],
                    rhs=rhs_t[:, slot * 512 : (slot + 1) * 512],
                    start=(n == 0),
                    stop=(n == NK - 1),
                )

            def emit_epilogue(qc):
                oT = resp.tile([H + 1, 512], f32, tag="oT", name=f"oT{qc}")
                nc.vector.tensor_copy(out=oT, in_=outqs[qc])
                nc.sync.dma_start(
                    out=out_d[:, qc * 512 : (qc + 1) * 512], in_=oT
                )

            def emit_score_pair(qc, p):
                """Two concurrent row-tiled matmuls for pair p (lo kt | hi kt)."""
                st = sc_state.setdefault(qc, [None] * len(GROUPS))
                for n in (2 * p, 2 * p + 1):
                    g = n // 3
                    lo, hi = GROUPS[g]
                    if st[g] is None:
                        st[g] = ps.tile(
                            [128, (hi - lo) * 512], f32, tag="sc", name=f"sc{qc}_{g}"
                        )
                    slot = n - lo
                    kt = _kt_of(n)
                    if n % 2 == 0:
                        lhsT = klo[:, kt * 128 : (kt + 1) * 128]
                        rhs = qk[:H, qc * 512 : (qc + 1) * 512]
                    else:
                        lhsT = qk[H:, kt * 128 : (kt + 1) * 128]
                        rhs = vq[H:, qc * 512 : (qc + 1) * 512]
                    nc.tensor.matmul(
                        st[g][:, slot * 512 : (slot + 1) * 512],
                        lhsT=lhsT,
                        rhs=rhs,
                        start=True,
                        stop=True,
                    )
                    if n == hi - 1:  # group full -> exp
                        cols = (hi - lo) * 512
                        if (qc, g) in DVE_GROUPS:
                            pt = ptp.tile(
                                [128, cols], i16, tag="pT", name=f"pt{qc}_{g}"
                            )
                            nc.vector.tensor_scalar(
                                out=pt,
                                in0=st[g],
                                scalar1=SCH_A,
                                scalar2=SCH_C,
                                op0=Mult,
                                op1=Add,
                            )
                        else:
                            pt = ptp.tile(
                                [128, cols], bf16, tag="pT", name=f"pt{qc}_{g}"
                            )
                            nc.scalar.activation(
                                out=pt, in_=st[g], func=Exp, scale=SCALE
                            )
                        pt_tiles[(qc, g)] = pt

            # ---- projection phase (chunk c), with q-chunk-0 scores inline ----
            for c in range(4):
                cc = slice(c * 512, (c + 1) * 512)

                def proj(w, nm, c0=c, halves=(0, 1)):
                    p = pj_tile(256 * len(halves), f"ps{nm}{c0}")
                    for hh in halves:
                        for dt in range(DT):
                            nc.tensor.matmul(
                                p[:, hh * 256 : (hh + 1) * 256],
                                lhsT=w[:, dt, :],
                                rhs=xT[:, c0, hh, dt * 256 : (dt + 1) * 256],
                                start=(dt == 0),
                                stop=(dt == DT - 1),
                            )
                    return p

                psA = proj(wA, "A")
                nc.vector.tensor_scalar_add(out=vq[:, cc], in0=psA, scalar1=bA)
                psB = proj(wB, "B")
                nc.vector.tensor_scalar_add(out=qk[:, cc], in0=psB, scalar1=bB)
                psC = proj(wC, "C", halves=(0,))
                nc.vector.tensor_scalar_add(
                    out=klo[:, c * 512 : c * 512 + 256],
                    in0=psC[:H, :],
                    scalar1=bC[:H, :],
                )
                # V rows -> v65 [k, h] layout
                for j in range(4):
                    kt = 4 * c + j
                    if USE_DMA_TRANSPOSE:
                        nc.sync.dma_start(
                            out=v65[:, kt, :H],
                            in_=vq[:H, kt * 128 : (kt + 1) * 128],
                            transpose=True,
                        )
                    else:
                        tp = pj_tile(128, f"vtr{kt}", dtype=bf16)
                        nc.tensor.transpose(
                            tp[:, :H],
                            vq[:H, kt * 128 : (kt + 1) * 128],
                            identb[:H, :H],
                        )
                        nc.vector.tensor_copy(out=v65[:, kt, :H], in_=tp[:, :H])
                # q-chunk 0 scores for this chunk's two pairs
                emit_score_pair(0, 2 * c)
                emit_score_pair(0, 2 * c + 1)

            # ---- steady phase ----
            # PV(qc) trails exp(qc) by one group within the same q-chunk;
            # PV(0) (whose outq can't be allocated mid-projection) is pumped
            # during qc=1. outq banks alternate between tags "o" and "pj".
            pv_cursor = [0] * NQ

            def pump_pv(qc, limit_n):
                while pv_cursor[qc] < min(limit_n, NK):
                    emit_pv(qc, pv_cursor[qc])
                    pv_cursor[qc] += 1

            outqs[0] = ps.tile([H + 1, 512], f32, tag="o", name="outq0", bufs=1)
            for qc in range(1, NQ):
                for p in range(8):
                    emit_score_pair(qc, p)
                    if qc == 1:
                        pump_pv(0, 2 * p + 2)
                    ge = sum(1 for g in range(6) if (qc, g) in pt_tiles)
                    if ge >= 2:
                        if outqs[qc] is None:
                            outqs[qc] = ps.tile(
                                [H + 1, 512],
                                f32,
                                tag=("o", "pj")[qc % 2],
                                name=f"outq{qc}",
                                bufs=1,
                            )
                        pump_pv(qc, 3 * (ge - 1))
                if qc == 1:
                    pump_pv(0, NK)
                    emit_epilogue(0)
                pump_pv(qc, NK)
                emit_epilogue(qc)

    nc.compile()
    return nc


def _get_nc():
    if "nc" not in _cache:
        _cache["nc"] = _build()
    return _cache["nc"]


def _prep_inputs(x, Wq, bq, Wk, bk, Wv, bv):
    import ml_dtypes

    x = np.asarray(x, np.float32)
    Wq = np.asarray(Wq, np.float32)
    Wk = np.asarray(Wk, np.float32)
    Wv = np.asarray(Wv, np.float32)
    bq = np.asarray(bq, np.float32).ravel()
    bk = np.asarray(bk, np.float32).ravel()
    bv = np.asarray(bv, np.float32).ravel()

    def wprep(w):  # [768,128] -> [128, DT*128]: (p, dt*128+h) = w[dt*128+p, h]
        return np.ascontiguousarray(
            w.reshape(DT, 128, 128).transpose(1, 0, 2).reshape(128, DT * 128)
        ).astype(ml_dtypes.bfloat16)

    z = np.zeros((D, H), np.float32)
    common = {
        "wA": wprep(np.concatenate([Wv, Wq], axis=1)),
        "wB": wprep(np.concatenate([Wq, Wk], axis=1)),
        "wC": wprep(np.concatenate([Wk, z], axis=1)),
        "bA": np.ascontiguousarray(np.concatenate([bv, bq]).reshape(128, 1)),
        "bB": np.ascontiguousarray(np.concatenate([bq, bk]).reshape(128, 1)),
        "bC": np.ascontiguousarray(
            np.concatenate([bk, np.zeros(H, np.float32)]).reshape(128, 1)
        ),
    }
    return x, common


def _xprep(xb):
    """[S, D] f32 -> [128, NQ, 2, DT*256] bf16:
    (p, c, h, dt*256+j) = x[c*512 + h*256 + j, dt*128 + p]"""
    import ml_dtypes

    t = xb.reshape(NQ, 2, 256, DT, 128).transpose(4, 0, 1, 3, 2)
    return np.ascontiguousarray(t.reshape(128, NQ, 2, DT * 256)).astype(
        ml_dtypes.bfloat16
    )


def _unshard_out(o):
    """[65, NQ*512] out^T with denominator row -> [S, H]"""
    o = np.asarray(o, np.float32)
    return (o[:H, :] / o[H : H + 1, :]).T


def _in_maps(x, common):
    return [{"xp": _xprep(x[b]), **common} for b in range(B)]


def kernel(x, Wq, bq, Wk, bk, Wv, bv, **_):
    from concourse.bass_utils import run_bass_kernel_spmd

    nc = _get_nc()
    x, common = _prep_inputs(x, Wq, bq, Wk, bk, Wv, bv)
    res = run_bass_kernel_spmd(nc, _in_maps(x, common), core_ids=list(range(B)))
    return np.stack([_unshard_out(res.results[b]["out"]) for b in range(B)])


# revision 26
# speedup vs baseline: 1.0765x; 1.0350x over previous
"""Single-head attention on 8 Trainium2 NeuronCores, batch-sharded.

Per core (one batch element b). Host-side layouts make every DMA a large
contiguous read (6KB/partition for x).

Projections (bf16, chunk order 0..3, interleaved with everything else):
  A [Wv|Wq] -> vq tile:  rows 0-63 v^T, rows 64-127 q^T (hi copy)
  B [Wq|Wk] -> qk tile:  rows 0-63 q^T (lo copy), rows 64-127 k^T (hi tiles)
  C [Wk|0]  -> klo tile: rows 0-63 k^T (lo tiles), N=256 per chunk
Within each 512-col chunk c, k-tiles 4c,4c+1 are assigned to the LOW
partition half and 4c+2,4c+3 to the HIGH half, so paired scores can start
right after chunk 0 arrives. Projection psums alternate between the "pj"
and "o" PSUM banks so consecutive groups pipeline (the WAR wait on the
bias-add is covered by the other group's matmuls).

Scores (bf16, ~2x via PE row tiling): each pair (lo k-tile | hi k-tile)
runs as two CONCURRENT K=64 matmuls in array row groups 0-63 / 64-127
(tile_position auto-derived from base partitions). 3 matmuls fill a
[128,1536] psum tile (2 such tiles rotate = 6 banks).

exp: split across TWO engines. ACT handles q-chunk 0 and groups 1,3,5 of
q-chunks 1-3 (exact exp, scale=1/8 folded in, bf16 out). DVE handles
groups 0,2,4 of q-chunks 1-3 with a Schraudolph bit-trick: bf16 bits of
2^y are linear in y, so   bits = round(s_raw * (log2e/8 * 128) + 16248.5)
computed by ONE tensor_scalar (mult+add, f32 psum in, int16 out) IS
exp(s/8) to within ~2% — the int16 tile is bitcast to bf16 for the PV
matmul. Splitting exp removes it as the serial bottleneck (sim rel err
with this mix: 0.84e-2 < 2e-2).

PV (bf16): per k-tile matmul, M=65 (V plus a ones row -> softmax
denominator), accumulated over the 16 k-tiles into a 1-bank psum, PV of
group g trailing exp of group g+1 within the same q-chunk.

V layout: V^T rows of vq are DMA-TRANSPOSED (SBUF->SBUF XBAR) straight
into v65 [128, kt, 68] — no PE transposes anywhere in the kernel.

Epilogue per q-chunk: DVE copy psum->SBUF, DMA out^T [65,512] f32; the
host does the divide-by-denominator and the final transpose (cheap).
"""

import numpy as np

USE_DMA_TRANSPOSE = False

B, S, D, H = 8, 2048, 768, 64
DT = D // 128          # 6 d-tiles
NQ = S // 512          # 4 q-chunks of 512
NK = S // 128          # 16 k-tiles of 128
SCALE = 1.0 / np.sqrt(H).item()
SCH_A = SCALE * np.log2(np.e).item() * 128.0   # Schraudolph slope
SCH_C = 16248.5                                 # Schraudolph offset (tuned)
GROUPS = ((0, 3), (3, 6), (6, 9), (9, 12), (12, 15), (15, 16))
DVE_GROUPS = {(qc, g) for qc in (1, 2, 3) for g in (0, 2, 4)}

_cache = {}


def _kt_of(n):
    """Score-matmul n (0..15) within a q-chunk -> k-tile index."""
    p, e = divmod(n, 2)
    c, i = divmod(p, 2)
    return 4 * c + i + 2 * e


def _build():
    import concourse.mybir as mybir
    import concourse.tile as tile
    from concourse import bacc

    f32 = mybir.dt.float32
    bf16 = mybir.dt.bfloat16
    i16 = mybir.dt.int16
    Exp = mybir.ActivationFunctionType.Exp
    Mult = mybir.AluOpType.mult
    Add = mybir.AluOpType.add

    from concourse.masks import make_identity

    nc = bacc.Bacc(None)
    xp_d = nc.dram_tensor("xp", [128, NQ, 2, DT * 256], bf16, kind="ExternalInput")
    wA_d = nc.dram_tensor("wA", [128, DT * 128], bf16, kind="ExternalInput")
    wB_d = nc.dram_tensor("wB", [128, DT * 128], bf16, kind="ExternalInput")
    wC_d = nc.dram_tensor("wC", [128, DT * 128], bf16, kind="ExternalInput")
    bA_d = nc.dram_tensor("bA", [128, 1], f32, kind="ExternalInput")
    bB_d = nc.dram_tensor("bB", [128, 1], f32, kind="ExternalInput")
    bC_d = nc.dram_tensor("bC", [128, 1], f32, kind="ExternalInput")
    out_d = nc.dram_tensor("out", [H + 1, NQ * 512], f32, kind="ExternalOutput")

    with tile.TileContext(nc) as tc:
        with (
            tc.tile_pool(name="big", bufs=1) as big,
            tc.tile_pool(name="small", bufs=1) as small,
            tc.tile_pool(name="pt", bufs=10) as ptp,
            tc.tile_pool(name="res", bufs=2) as resp,
            tc.tile_pool(name="ps", bufs=2, space="PSUM") as ps,
        ):
            if not USE_DMA_TRANSPOSE:
                ident = small.tile([128, 128], f32)
                make_identity(nc, ident)
                identb = small.tile([128, 128], bf16)
                nc.gpsimd.tensor_copy(out=identb, in_=ident)

            # warm the ACT exp table during DMA fill
            zwarm = small.tile([128, 8], f32)
            nc.gpsimd.memset(zwarm, 0.0)
            wwarm = small.tile([128, 8], bf16)
            nc.scalar.activation(out=wwarm, in_=zwarm, func=Exp)

            # ---- DMAs, ordered so chunk-0 compute starts earliest ----
            wA = small.tile([128, DT, 128], bf16)
            wB = small.tile([128, DT, 128], bf16)
            wC = small.tile([128, DT, 128], bf16)
            bA = small.tile([128, 1], f32)
            bB = small.tile([128, 1], f32)
            bC = small.tile([128, 1], f32)
            xT = big.tile([128, NQ, 2, DT * 256], bf16)

            nc.sync.dma_start(out=wA, in_=wA_d[:, :].rearrange("p (t h) -> p t h", t=DT))
            nc.sync.dma_start(out=bA, in_=bA_d[:, :])
            nc.sync.dma_start(out=xT[:, 0, 0, :], in_=xp_d[:, 0, 0, :])
            nc.sync.dma_start(out=xT[:, 0, 1, :], in_=xp_d[:, 0, 1, :])
            nc.sync.dma_start(out=wB, in_=wB_d[:, :].rearrange("p (t h) -> p t h", t=DT))
            nc.sync.dma_start(out=bB, in_=bB_d[:, :])
            nc.sync.dma_start(out=wC, in_=wC_d[:, :].rearrange("p (t h) -> p t h", t=DT))
            nc.sync.dma_start(out=bC, in_=bC_d[:, :])
            for c in (1, 2, 3):
                for hh in (0, 1):
                    nc.sync.dma_start(out=xT[:, c, hh, :], in_=xp_d[:, c, hh, :])

            # ---- persistent tensors ----
            vq = big.tile([128, S], bf16, tag="vq")    # v^T lo / q^T hi
            qk = big.tile([128, S], bf16, tag="qk")    # q^T lo / k^T hi
            klo = big.tile([H, S], bf16, tag="klo")    # k^T lo tiles
            v65 = big.tile([128, NK, H + 4], bf16, tag="v65")
            nc.gpsimd.memset(v65[:, :, H : H + 1], 1.0)

            # proj psums alternate between two 1-bank tags
            alt = {"n": 0}

            def pj_tile(cols, nm, dtype=f32):
                tag = ("pj", "o")[alt["n"] % 2]
                alt["n"] += 1
                return ps.tile([128, cols], dtype, tag=tag, name=nm, bufs=1)

            pt_tiles = {}
            sc_state = {}
            outqs = [None] * NQ

            def emit_pv(qc, n):
                g = n // 3
                lo, _hi = GROUPS[g]
                pt = pt_tiles[(qc, g)]
                rhs_t = pt if pt.dtype == bf16 else pt.bitcast(bf16)
                slot = n - lo
                nc.tensor.matmul(
                    outqs[qc],
                    lhsT=v65[:, _kt_of(n), : H + 1],
                    rhs=rhs_t[:, slot * 512 : (slot + 1) * 512],
                    start=(n == 0),
                    stop=(n == NK - 1),
                )

            def emit_epilogue(qc):
                oT = resp.tile([H + 1, 512], f32, tag="oT", name=f"oT{qc}")
                nc.vector.tensor_copy(out=oT, in_=outqs[qc])
                nc.sync.dma_start(
                    out=out_d[:, qc * 512 : (qc + 1) * 512], in_=oT
                )

            def emit_score_pair(qc, p):
                """Two concurrent row-tiled matmuls for pair p (lo kt | hi kt)."""
                st = sc_state.setdefault(qc, [None] * len(GROUPS))
                for n in (2 * p, 2 * p + 1):
                    g = n // 3
                    lo, hi = GROUPS[g]
                    if st[g] is None:
                        st[g] = ps.tile(
                            [128, (hi - lo) * 512], f32, tag="sc", name=f"sc{qc}_{g}"
                        )
                    slot = n - lo
                    kt = _kt_of(n)
                    if n % 2 == 0:
                        lhsT = klo[:, kt * 128 : (kt + 1) * 128]
                        rhs = qk[:H, qc * 512 : (qc + 1) * 512]
                    else:
                        lhsT = qk[H:, kt * 128 : (kt + 1) * 128]
                        rhs = vq[H:, qc * 512 : (qc + 1) * 512]
                    nc.tensor.matmul(
                        st[g][:, slot * 512 : (slot + 1) * 512],
                        lhsT=lhsT,
                        rhs=rhs,
                        start=True,
                        stop=True,
                    )
                    if n == hi - 1:  # group full -> exp
                        cols = (hi - lo) * 512
                        if (qc, g) in DVE_GROUPS:
                            pt = ptp.tile(
                                [128, cols], i16, tag="pT", name=f"pt{qc}_{g}"
                            )
                            nc.vector.tensor_scalar(
                                out=pt,
                                in0=st[g],
                                scalar1=SCH_A,
                                scalar2=SCH_C,
                                op0=Mult,
                                op1=Add,
                            )
                        else:
                            pt = ptp.tile(
                                [128, cols], bf16, tag="pT", name=f"pt{qc}_{g}"
                            )
                            nc.scalar.activation(
                                out=pt, in_=st[g], func=Exp, scale=SCALE
                            )
                        pt_tiles[(qc, g)] = pt

            # ---- projection phase (chunk c), with q-chunk-0 scores inline ----
            for c in range(4):
                cc = slice(c * 512, (c + 1) * 512)

                def proj(w, nm, c0=c, halves=(0, 1)):
                    p = pj_tile(256 * len(halves), f"ps{nm}{c0}")
                    for hh in halves:
                        for dt in range(DT):
                            nc.tensor.matmul(
                                p[:, hh * 256 : (hh + 1) * 256],
                                lhsT=w[:, dt, :],
                                rhs=xT[:, c0, hh, dt * 256 : (dt + 1) * 256],
                                start=(dt == 0),
                                stop=(dt == DT - 1),
                            )
                    return p

                psA = proj(wA, "A")
                nc.vector.tensor_scalar_add(out=vq[:, cc], in0=psA, scalar1=bA)
                psB = proj(wB, "B")
                nc.vector.tensor_scalar_add(out=qk[:, cc], in0=psB, scalar1=bB)
                psC = proj(wC, "C", halves=(0,))
                nc.vector.tensor_scalar_add(
                    out=klo[:, c * 512 : c * 512 + 256],
                    in0=psC[:H, :],
                    scalar1=bC[:H, :],
                )
                # V rows -> v65 [k, h] layout
                for j in range(4):
                    kt = 4 * c + j
                    if USE_DMA_TRANSPOSE:
                        nc.sync.dma_start(
                            out=v65[:, kt, :H],
                            in_=vq[:H, kt * 128 : (kt + 1) * 128],
                            transpose=True,
                        )
                    else:
                        tp = pj_tile(128, f"vtr{kt}", dtype=bf16)
                        nc.tensor.transpose(
                            tp[:, :H],
                            vq[:H, kt * 128 : (kt + 1) * 128],
                            identb[:H, :H],
                        )
                        nc.vector.tensor_copy(out=v65[:, kt, :H], in_=tp[:, :H])
                # q-chunk 0 scores for this chunk's two pairs
                emit_score_pair(0, 2 * c)
                emit_score_pair(0, 2 * c + 1)

            # ---- steady phase ----
            # PV(qc-1) runs one q-chunk behind scores(qc) (no exp-wait
            # stalls); only PV(3) additionally trails its own exps by two
            # groups during qc=3 to shorten the kernel tail. outq banks
            # alternate between the "o" and "pj" tags.
            pv_cursor = [0] * NQ

            def pump_pv(qc, limit_n):
                while pv_cursor[qc] < min(limit_n, NK):
                    emit_pv(qc, pv_cursor[qc])
                    pv_cursor[qc] += 1

            outqs[0] = ps.tile([H + 1, 512], f32, tag="o", name="outq0", bufs=1)
            for qc in range(1, NQ):
                for p in range(8):
                    emit_score_pair(qc, p)
                    pump_pv(qc - 1, 2 * p + 2)
                    if qc == NQ - 1:
                        ge = sum(1 for g in range(6) if (qc, g) in pt_tiles)
                        if ge >= 3:
                            if outqs[qc] is None:
                                outqs[qc] = ps.tile(
                                    [H + 1, 512],
                                    f32,
                                    tag=("o", "pj")[qc % 2],
                                    name=f"outq{qc}",
                                    bufs=1,
                                )
                            pump_pv(qc, 3 * (ge - 2))
                emit_epilogue(qc - 1)
                if qc < NQ - 1:
                    outqs[qc] = ps.tile(
                        [H + 1, 512],
                        f32,
                        tag=("o", "pj")[qc % 2],
                        name=f"outq{qc}",
                        bufs=1,
                    )
            pump_pv(NQ - 1, NK)
            emit_epilogue(NQ - 1)

    nc.compile()
    return nc


def _get_nc():
    if "nc" not in _cache:
        _cache["nc"] = _build()
    return _cache["nc"]


def _prep_inputs(x, Wq, bq, Wk, bk, Wv, bv):
    import ml_dtypes

    x = np.asarray(x, np.float32)
    Wq = np.asarray(Wq, np.float32)
    Wk = np.asarray(Wk, np.float32)
    Wv = np.asarray(Wv, np.float32)
    bq = np.asarray(bq, np.float32).ravel()
    bk = np.asarray(bk, np.float32).ravel()
    bv = np.asarray(bv, np.float32).ravel()

    def wprep(w):  # [768,128] -> [128, DT*128]: (p, dt*128+h) = w[dt*128+p, h]
        return np.ascontiguousarray(
            w.reshape(DT, 128, 128).transpose(1, 0, 2).reshape(128, DT * 128)
        ).astype(ml_dtypes.bfloat16)

    z = np.zeros((D, H), np.float32)
    common = {
        "wA": wprep(np.concatenate([Wv, Wq], axis=1)),
        "wB": wprep(np.concatenate([Wq, Wk], axis=1)),
        "wC": wprep(np.concatenate([Wk, z], axis=1)),
        "bA": np.ascontiguousarray(np.concatenate([bv, bq]).reshape(128, 1)),
        "bB": np.ascontiguousarray(np.concatenate([bq, bk]).reshape(128, 1)),
        "bC": np.ascontiguousarray(
            np.concatenate([bk, np.zeros(H, np.float32)]).reshape(128, 1)
        ),
    }
    return x, common


def _xprep(xb):
    """[S, D] f32 -> [128, NQ, 2, DT*256] bf16:
    (p, c, h, dt*256+j) = x[c*512 + h*256 + j, dt*128 + p]"""
    import ml_dtypes

    t = xb.reshape(NQ, 2, 256, DT, 128).transpose(4, 0, 1, 3, 2)
    return np.ascontiguousarray(t.reshape(128, NQ, 2, DT * 256)).astype(
        ml_dtypes.bfloat16
    )


def _unshard_out(o):
    """[65, NQ*512] out^T with denominator row -> [S, H]"""
    o = np.asarray(o, np.float32)
    return (o[:H, :] / o[H : H + 1, :]).T


def _in_maps(x, common):
    return [{"xp": _xprep(x[b]), **common} for b in range(B)]


def kernel(x, Wq, bq, Wk, bk, Wv, bv, **_):
    from concourse.bass_utils import run_bass_kernel_spmd

    nc = _get_nc()
    x, common = _prep_inputs(x, Wq, bq, Wk, bk, Wv, bv)
    res = run_bass_kernel_spmd(nc, _in_maps(x, common), core_ids=list(range(B)))
    return np.stack([_unshard_out(res.results[b]["out"]) for b in range(B)])


# revision 32
# speedup vs baseline: 1.0896x; 1.0122x over previous
"""Single-head attention on 8 Trainium2 NeuronCores, batch-sharded.

Per core (one batch element b). Host-side layouts make every DMA a large
contiguous read (6KB/partition for x).

Projections (bf16, chunk order 0..3, interleaved with everything else):
  A [Wv|Wq] -> vq tile:  rows 0-63 v^T, rows 64-127 q^T (hi copy)
  B [Wq|Wk] -> qk tile:  rows 0-63 q^T (lo copy), rows 64-127 k^T (hi tiles)
  C [Wk|0]  -> klo tile: rows 0-63 k^T (lo tiles), N=256 per chunk
Within each 512-col chunk c, k-tiles 4c,4c+1 are assigned to the LOW
partition half and 4c+2,4c+3 to the HIGH half, so paired scores can start
right after chunk 0 arrives. Projection psums alternate between the "pj"
and "o" PSUM banks so consecutive groups pipeline (the WAR wait on the
bias-add is covered by the other group's matmuls).

Scores (bf16, ~2x via PE row tiling): each pair (lo k-tile | hi k-tile)
runs as two CONCURRENT K=64 matmuls in array row groups 0-63 / 64-127
(tile_position auto-derived from base partitions). 3 matmuls fill a
[128,1536] psum tile (2 such tiles rotate = 6 banks).

exp: split across TWO engines. ACT handles q-chunk 0 and groups 1,3,5 of
q-chunks 1-3 (exact exp, scale=1/8 folded in, bf16 out). DVE handles
groups 0,2,4 of q-chunks 1-3 with a Schraudolph bit-trick: bf16 bits of
2^y are linear in y, so   bits = round(s_raw * (log2e/8 * 128) + 16248.5)
computed by ONE tensor_scalar (mult+add, f32 psum in, int16 out) IS
exp(s/8) to within ~2% — the int16 tile is bitcast to bf16 for the PV
matmul. Splitting exp removes it as the serial bottleneck (sim rel err
with this mix: 0.84e-2 < 2e-2).

PV (bf16): per k-tile matmul, M=65 (V plus a ones row -> softmax
denominator), accumulated over the 16 k-tiles into a 1-bank psum, PV of
group g trailing exp of group g+1 within the same q-chunk.

V layout: V^T rows of vq are DMA-TRANSPOSED (SBUF->SBUF XBAR) straight
into v65 [128, kt, 68] — no PE transposes anywhere in the kernel.

Epilogue per q-chunk: DVE copy psum->SBUF, DMA out^T [65,512] f32; the
host does the divide-by-denominator and the final transpose (cheap).
"""

import numpy as np

USE_DMA_TRANSPOSE = False

B, S, D, H = 8, 2048, 768, 64
DT = D // 128          # 6 d-tiles
NQ = S // 512          # 4 q-chunks of 512
NK = S // 128          # 16 k-tiles of 128
SCALE = 1.0 / np.sqrt(H).item()
SCH_A = SCALE * np.log2(np.e).item() * 128.0   # Schraudolph slope
SCH_C = 16248.5                                 # Schraudolph offset (tuned)
GROUPS = ((0, 3), (3, 6), (6, 9), (9, 12), (12, 15), (15, 16))
DVE_GROUPS = {(qc, g) for qc in (1, 2, 3) for g in (0, 2, 4)}

_cache = {}


def _kt_of(n):
    """Score-matmul n (0..15) within a q-chunk -> k-tile index."""
    p, e = divmod(n, 2)
    c, i = divmod(p, 2)
    return 4 * c + i + 2 * e


def _build():
    import concourse.mybir as mybir
    import concourse.tile as tile
    from concourse import bacc

    f32 = mybir.dt.float32
    bf16 = mybir.dt.bfloat16
    i16 = mybir.dt.int16
    Exp = mybir.ActivationFunctionType.Exp
    Mult = mybir.AluOpType.mult
    Add = mybir.AluOpType.add

    from concourse.masks import make_identity

    nc = bacc.Bacc(None)
    xp_d = nc.dram_tensor("xp", [128, NQ, 2, DT * 256], bf16, kind="ExternalInput")
    wA_d = nc.dram_tensor("wA", [128, DT * 128], bf16, kind="ExternalInput")
    wB_d = nc.dram_tensor("wB", [128, DT * 128], bf16, kind="ExternalInput")
    wC_d = nc.dram_tensor("wC", [128, DT * 64], bf16, kind="ExternalInput")
    bA_d = nc.dram_tensor("bA", [128, 1], f32, kind="ExternalInput")
    bB_d = nc.dram_tensor("bB", [128, 1], f32, kind="ExternalInput")
    bC_d = nc.dram_tensor("bC", [128, 1], f32, kind="ExternalInput")
    out_d = nc.dram_tensor("out", [H + 1, NQ * 512], f32, kind="ExternalOutput")

    with tile.TileContext(nc) as tc:
        with (
            tc.tile_pool(name="big", bufs=1) as big,
            tc.tile_pool(name="small", bufs=1) as small,
            tc.tile_pool(name="pt", bufs=10) as ptp,
            tc.tile_pool(name="res", bufs=2) as resp,
            tc.tile_pool(name="ps", bufs=2, space="PSUM") as ps,
        ):
            if not USE_DMA_TRANSPOSE:
                ident = small.tile([128, 128], f32)
                make_identity(nc, ident)
                identb = small.tile([128, 128], bf16)
                nc.gpsimd.tensor_copy(out=identb, in_=ident)

            # warm the ACT exp table during DMA fill
            zwarm = small.tile([128, 8], f32)
            nc.gpsimd.memset(zwarm, 0.0)
            wwarm = small.tile([128, 8], bf16)
            nc.scalar.activation(out=wwarm, in_=zwarm, func=Exp)

            # ---- DMAs, ordered so chunk-0 compute starts earliest ----
            wA = small.tile([128, DT, 128], bf16)
            wB = small.tile([128, DT, 128], bf16)
            wC = small.tile([128, DT, H], bf16)
            bA = small.tile([128, 1], f32)
            bB = small.tile([128, 1], f32)
            bC = small.tile([128, 1], f32)
            xT = big.tile([128, NQ, 2, DT * 256], bf16)

            nc.sync.dma_start(out=wA, in_=wA_d[:, :].rearrange("p (t h) -> p t h", t=DT))
            nc.sync.dma_start(out=bA, in_=bA_d[:, :])
            nc.sync.dma_start(out=xT[:, 0, 0, :], in_=xp_d[:, 0, 0, :])
            nc.sync.dma_start(out=xT[:, 0, 1, :], in_=xp_d[:, 0, 1, :])
            nc.sync.dma_start(out=wB, in_=wB_d[:, :].rearrange("p (t h) -> p t h", t=DT))
            nc.sync.dma_start(out=xT[:, 1, 0, :], in_=xp_d[:, 1, 0, :])
            nc.sync.dma_start(out=xT[:, 1, 1, :], in_=xp_d[:, 1, 1, :])
            nc.sync.dma_start(out=bB, in_=bB_d[:, :])
            nc.sync.dma_start(out=wC, in_=wC_d[:, :].rearrange("p (t h) -> p t h", t=DT))
            nc.sync.dma_start(out=bC, in_=bC_d[:, :])
            for c in (2, 3):
                for hh in (0, 1):
                    nc.sync.dma_start(out=xT[:, c, hh, :], in_=xp_d[:, c, hh, :])

            # ---- persistent tensors ----
            vq = big.tile([128, S], bf16, tag="vq")    # v^T lo / q^T hi
            qk = big.tile([128, S], bf16, tag="qk")    # q^T lo / k^T hi
            klo = big.tile([H, S], bf16, tag="klo")    # k^T lo tiles
            v65 = big.tile([128, NK, H + 4], bf16, tag="v65")
            nc.gpsimd.memset(v65[:, :, H : H + 1], 1.0)

            # proj psums alternate between two 1-bank tags
            alt = {"n": 0}

            def pj_tile(cols, nm, dtype=f32):
                tag = ("pj", "o")[alt["n"] % 2]
                alt["n"] += 1
                return ps.tile([128, cols], dtype, tag=tag, name=nm, bufs=1)

            pt_tiles = {}
            sc_state = {}
            outqs = [None] * NQ

            def emit_pv(qc, n):
                g = n // 3
                lo, _hi = GROUPS[g]
                pt = pt_tiles[(qc, g)]
                rhs_t = pt if pt.dtype == bf16 else pt.bitcast(bf16)
                slot = n - lo
                nc.tensor.matmul(
                    outqs[qc],
                    lhsT=v65[:, _kt_of(n), : H + 1],
                    rhs=rhs_t[:, slot * 512 : (slot + 1) * 512],
                    start=(n == 0),
                    stop=(n == NK - 1),
                )

            def emit_epilogue(qc):
                oT = resp.tile([H + 1, 512], f32, tag="oT", name=f"oT{qc}")
                nc.vector.tensor_copy(out=oT, in_=outqs[qc])
                nc.sync.dma_start(
                    out=out_d[:, qc * 512 : (qc + 1) * 512], in_=oT
                )

            def emit_score_pair(qc, p):
                """Two concurrent row-tiled matmuls for pair p (lo kt | hi kt)."""
                st = sc_state.setdefault(qc, [None] * len(GROUPS))
                for n in (2 * p, 2 * p + 1):
                    g = n // 3
                    lo, hi = GROUPS[g]
                    if st[g] is None:
                        st[g] = ps.tile(
                            [128, (hi - lo) * 512], f32, tag="sc", name=f"sc{qc}_{g}"
                        )
                    slot = n - lo
                    kt = _kt_of(n)
                    if n % 2 == 0:
                        lhsT = klo[:, kt * 128 : (kt + 1) * 128]
                        rhs = qk[:H, qc * 512 : (qc + 1) * 512]
                    else:
                        lhsT = qk[H:, kt * 128 : (kt + 1) * 128]
                        rhs = vq[H:, qc * 512 : (qc + 1) * 512]
                    nc.tensor.matmul(
                        st[g][:, slot * 512 : (slot + 1) * 512],
                        lhsT=lhsT,
                        rhs=rhs,
                        start=True,
                        stop=True,
                    )
                    if n == hi - 1:  # group full -> exp
                        cols = (hi - lo) * 512
                        if (qc, g) in DVE_GROUPS:
                            pt = ptp.tile(
                                [128, cols], i16, tag="pT", name=f"pt{qc}_{g}"
                            )
                            nc.vector.tensor_scalar(
                                out=pt,
                                in0=st[g],
                                scalar1=SCH_A,
                                scalar2=SCH_C,
                                op0=Mult,
                                op1=Add,
                            )
                        else:
                            pt = ptp.tile(
                                [128, cols], bf16, tag="pT", name=f"pt{qc}_{g}"
                            )
                            nc.scalar.activation(
                                out=pt, in_=st[g], func=Exp, scale=SCALE
                            )
                        pt_tiles[(qc, g)] = pt

            # ---- projection phase (chunk c), with q-chunk-0 scores inline ----
            for c in range(4):
                cc = slice(c * 512, (c + 1) * 512)

                def proj(w, nm, c0=c, halves=(0, 1)):
                    p = pj_tile(256 * len(halves), f"ps{nm}{c0}")
                    m = w.shape[-1]
                    for hh in halves:
                        for dt in range(DT):
                            nc.tensor.matmul(
                                p[:m, hh * 256 : (hh + 1) * 256],
                                lhsT=w[:, dt, :],
                                rhs=xT[:, c0, hh, dt * 256 : (dt + 1) * 256],
                                start=(dt == 0),
                                stop=(dt == DT - 1),
                            )
                    return p

                psA = proj(wA, "A")
                nc.vector.tensor_scalar_add(out=vq[:, cc], in0=psA, scalar1=bA)
                psB = proj(wB, "B")
                nc.vector.tensor_scalar_add(out=qk[:, cc], in0=psB, scalar1=bB)
                psC = proj(wC, "C", halves=(0,))
                nc.vector.tensor_scalar_add(
                    out=klo[:, c * 512 : c * 512 + 256],
                    in0=psC[:H, :],
                    scalar1=bC[:H, :],
                )
                # V rows -> v65 [k, h] layout
                for j in range(4):
                    kt = 4 * c + j
                    if USE_DMA_TRANSPOSE:
                        nc.sync.dma_start(
                            out=v65[:, kt, :H],
                            in_=vq[:H, kt * 128 : (kt + 1) * 128],
                            transpose=True,
                        )
                    else:
                        tp = pj_tile(128, f"vtr{kt}", dtype=bf16)
                        nc.tensor.transpose(
                            tp[:, :H],
                            vq[:H, kt * 128 : (kt + 1) * 128],
                            identb[:H, :H],
                        )
                        nc.vector.tensor_copy(out=v65[:, kt, :H], in_=tp[:, :H])
                # q-chunk 0 scores for this chunk's two pairs
                emit_score_pair(0, 2 * c)
                emit_score_pair(0, 2 * c + 1)

            # ---- steady phase ----
            # PV(qc-1) runs one q-chunk behind scores(qc) (no exp-wait
            # stalls); only PV(3) additionally trails its own exps by two
            # groups during qc=3 to shorten the kernel tail. outq banks
            # alternate between the "o" and "pj" tags.
            pv_cursor = [0] * NQ

            def pump_pv(qc, limit_n):
                while pv_cursor[qc] < min(limit_n, NK):
                    emit_pv(qc, pv_cursor[qc])
                    pv_cursor[qc] += 1

            outqs[0] = ps.tile([H + 1, 512], f32, tag="o", name="outq0", bufs=1)
            for qc in range(1, NQ):
                for p2 in range(0, 8, 2):
                    emit_score_pair(qc, p2)
                    emit_score_pair(qc, p2 + 1)
                    pump_pv(qc - 1, 2 * p2 + 4)
                    if qc == NQ - 1:
                        ge = sum(1 for g in range(6) if (qc, g) in pt_tiles)
                        if ge >= 3:
                            if outqs[qc] is None:
                                outqs[qc] = ps.tile(
                                    [H + 1, 512],
                                    f32,
                                    tag=("o", "pj")[qc % 2],
                                    name=f"outq{qc}",
                                    bufs=1,
                                )
                            pump_pv(qc, 3 * (ge - 2))
                emit_epilogue(qc - 1)
                if qc < NQ - 1:
                    outqs[qc] = ps.tile(
                        [H + 1, 512],
                        f32,
                        tag=("o", "pj")[qc % 2],
                        name=f"outq{qc}",
                        bufs=1,
                    )
            pump_pv(NQ - 1, NK)
            emit_epilogue(NQ - 1)

    nc.compile()
    return nc


def _get_nc():
    if "nc" not in _cache:
        _cache["nc"] = _build()
    return _cache["nc"]


def _prep_inputs(x, Wq, bq, Wk, bk, Wv, bv):
    import ml_dtypes

    x = np.asarray(x, np.float32)
    Wq = np.asarray(Wq, np.float32)
    Wk = np.asarray(Wk, np.float32)
    Wv = np.asarray(Wv, np.float32)
    bq = np.asarray(bq, np.float32).ravel()
    bk = np.asarray(bk, np.float32).ravel()
    bv = np.asarray(bv, np.float32).ravel()

    def wprep(w):  # [768,128] -> [128, DT*128]: (p, dt*128+h) = w[dt*128+p, h]
        return np.ascontiguousarray(
            w.reshape(DT, 128, 128).transpose(1, 0, 2).reshape(128, DT * 128)
        ).astype(ml_dtypes.bfloat16)

    def wprep64(w):  # [768,64] -> [128, DT*64]
        return np.ascontiguousarray(
            w.reshape(DT, 128, H).transpose(1, 0, 2).reshape(128, DT * H)
        ).astype(ml_dtypes.bfloat16)

    common = {
        "wA": wprep(np.concatenate([Wv, Wq], axis=1)),
        "wB": wprep(np.concatenate([Wq, Wk], axis=1)),
        "wC": wprep64(Wk),
        "bA": np.ascontiguousarray(np.concatenate([bv, bq]).reshape(128, 1)),
        "bB": np.ascontiguousarray(np.concatenate([bq, bk]).reshape(128, 1)),
        "bC": np.ascontiguousarray(
            np.concatenate([bk, np.zeros(H, np.float32)]).reshape(128, 1)
        ),
    }
    return x, common


def _xprep(xb):
    """[S, D] f32 -> [128, NQ, 2, DT*256] bf16:
    (p, c, h, dt*256+j) = x[c*512 + h*256 + j, dt*128 + p]"""
    import ml_dtypes

    t = xb.reshape(NQ, 2, 256, DT, 128).transpose(4, 0, 1, 3, 2)
    return np.ascontiguousarray(t.reshape(128, NQ, 2, DT * 256)).astype(
        ml_dtypes.bfloat16
    )


def _unshard_out(o):
    """[65, NQ*512] out^T with denominator row -> [S, H]"""
    o = np.asarray(o, np.float32)
    return (o[:H, :] / o[H : H + 1, :]).T


def _in_maps(x, common):
    return [{"xp": _xprep(x[b]), **common} for b in range(B)]


def kernel(x, Wq, bq, Wk, bk, Wv, bv, **_):
    from concourse.bass_utils import run_bass_kernel_spmd

    nc = _get_nc()
    x, common = _prep_inputs(x, Wq, bq, Wk, bk, Wv, bv)
    res = run_bass_kernel_spmd(nc, _in_maps(x, common), core_ids=list(range(B)))
    return np.stack([_unshard_out(res.results[b]["out"]) for b in range(B)])
